# revision 26
# baseline (speedup 1.0000x reference)
"""
Trainium2 Bass kernel for nn_BaseDecoder (9x9 local cost volume / spatial
correlation, kernel_size=1):

    out[b, di*9+dj, y, x] = sum_c t1[b,c,y,x] * t2p[b,c,y+di,x+dj]

t1/t2: [4, 128, 128, 256] f32, out: [4, 81, 128, 256] f32, zero-padded t2.

Strategy
--------
8 cores = (batch 4) x (H halves 2), fully data parallel; each core gets its
t1 shard [128c, 64y, 256x] and a zero-padded t2 slab [128c, 72y, 264x]
(4-row/4-col halo baked in on host), so no collectives are needed.

Per (y, x-block-of-128): channels live on partitions, so the only engine
that can produce the 81 shifted dot-products at speed is the PE via a
*banded* matmul psum[x, w] = sum_c t1[c, x0+x] * t2slab[c, y+di, x0+w],
done as 3 float32r matmuls of N=3*136 (3 'di' rows each; one PSUM bank
each; f32r streams 1 cyc/col at N>=256 vs 4 for fp32, rel err ~1.5e-4).
The useful outputs are the 9 diagonals psum[x, x+dj] of each 128x136 band
-- inherently non-rectangular, which no lockstep engine (PE/DVE/ACT/DMA)
can extract.

The band is evacuated PSUM->SBUF on VectorE (x-block 0) and ScalarE
(x-block 1) in parallel, then GPSIMD `ap_gather` -- the one engine with
per-Q7-core addressing -- extracts the 16-partition-aligned sub-band
out24[x, di, j] = band[x, di, 16*(x//16) + j], j in [0,24), as d=8-element
blocks (per-index cost dominates: d=1 measured 3.4x slower).  This shrinks
1224 -> 216 useful floats per partition per (y, x-block).  Results DMA out
via the ScalarE HWDGE ring (inputs use the sync ring).  The remaining
within-core skew (j = (x%16)+dj) is a cheap numpy take_along_axis on host.

Deskew split (SPLIT_DESKEW=1, the shipped config): Pool `ap_gather`
handles x-block 0; for x-block 1 the 32-partition-aligned 40-wide windows
ARE rectangular per partition-quadrant, so VectorE/ScalarE extract them
with 4 sliced copies (host residual (x%32)+dj there).  Measured (R-delta
slope, 8-core SPMD on HW): ~217 us/core/sweep; HBM roofline for the
23 MB/core I/O is ~64 us.  Tried and rejected: extracting the quadrant
windows directly from PSUM with 8 small ops/row (KERNEL_V3=1) measures
286 us -- the TRN2 PSUM read-write bubble errata penalizes many small
PSUM-source ops, so the single big evacuation wins.  Remaining wall:
GPSIMD gather + imperfect Pool/DVE overlap (shared SBUF port); going
lower needs custom Q7 ucode.
"""

import os
import sys

sys.path.insert(0, "/opt/trn_rl_repo")

from contextlib import ExitStack

import numpy as np

import concourse.bacc as bacc
import concourse.bass as bass
import concourse.mybir as mybir
import concourse.tile as tile
from concourse.bass_utils import run_bass_kernel_spmd

MD = 4
D = 9  # patch size (9x9 displacements)
B, C, H, W = 4, 128, 128, 256
HSH = H // 2  # 64 rows per shard
T2R = HSH + 2 * MD  # 72 t2 slab rows
T2C = W + 2 * MD  # 264 t2 slab cols
NG = 3  # di-groups (3 di each)
BAND_W = 136  # x' window width per di (128 + 8)
BAND_N = NG * BAND_W  # 408 = matmul N (one PSUM bank)
GD = int(os.environ.get("KERNEL_GATHER_D", "8"))  # ap_gather inner block size
assert GD in (1, 8)
# with d=8: 27 useful block-indices (9 di x 3 blocks of 8), padded to 32 (%16)
# with d=1: 216 useful element-indices, padded to 224
NBLK = (D * 24) // GD  # useful indices
NIDX = 224 if GD == 1 else 32  # padded index count (%16 == 0)
NOUT = NIDX * GD  # gathered floats per partition per x-block
X1W = D * 40  # 360: 40-wide 32-aligned windows for the split-deskew x-block 1
SLOTW_SPLIT = NOUT + X1W  # 616
NUSE = D * 24  # 216 useful floats
YB = int(os.environ.get("KERNEL_YB", "8"))  # y rows per output DMA batch

F32 = mybir.dt.float32
I16 = mybir.dt.int16

# matmul input dtype: "f32" (exact, 4 cyc/col), "f32r" (fast fp32 path),
# "bf16" (fast, lossy)
MM_DTYPE = os.environ.get("KERNEL_MM_DTYPE", "f32r")
# internal whole-kernel repeat count (for HW timing via differencing)
REPEAT = int(os.environ.get("KERNEL_REPEAT", "1"))
# comma list of stages to drop, for cost-model ablation: mm,evac,gather,outdma,indma
ABLATE = set(filter(None, os.environ.get("KERNEL_ABLATE", "").split(",")))
# 1 = one ap_gather per y (both x-blocks from one band tile); 0 = per x-block
GBATCH = int(os.environ.get("KERNEL_GBATCH", "0"))
BAND_BUFS = int(os.environ.get("KERNEL_BAND_BUFS", "2"))
# 1 = Pool ap_gather deskews only x-block 0; DVE/ACT extract x-block 1 via
# 16-partition-sliced rectangular group copies (they have slack; Pool is the wall)
SPLIT_DESKEW = int(os.environ.get("KERNEL_SPLIT_DESKEW", "1"))
# 1 = v3: extract 32-aligned 40-wide quadrant windows DIRECTLY from PSUM on
# DVE(xb0)/ACT(xb1); no full-band evacuation, no GPSIMD at all
V3 = int(os.environ.get("KERNEL_V3", "0"))


def build_gidx() -> np.ndarray:
    """Per-Q7-core gather index lists for ap_gather, stored 'wrapped':
    unwrapped[i] = idxs[16k + i%16, i//16] for core k.  Index values are in
    units of GD-element blocks.  With GBATCH, indices cover both x-blocks
    (xb-major) of a [2, NG, BAND_N] band tile."""
    nidx_tot = NIDX * (2 if GBATCH else 1)
    idx = np.zeros((128, nidx_tot // 16), dtype=np.int16)
    for k in range(8):
        for i in range(nidx_tot):
            xb, ih = divmod(i, NIDX) if GBATCH else (0, i)
            if ih < NBLK:
                di, m = divmod(ih, 24 // GD)
                g, e = divmod(di, NG)
                val = (xb * NG * BAND_N + g * BAND_N + e * BAND_W + 16 * k) // GD + m
            else:
                val = 0
            idx[16 * k + (i % 16), i // 16] = val
    return idx


def build_program():
    nc = bacc.Bacc("TRN2")

    if MM_DTYPE == "bf16":
        mm_dt = mybir.dt.bfloat16
    elif MM_DTYPE == "f32r":
        mm_dt = mybir.dt.float32r
    else:
        mm_dt = F32
    in_dram_dt = mm_dt if MM_DTYPE == "f32r" else F32
    t1s = nc.declare_dram_parameter("t1s", [C, HSH, W], in_dram_dt, isOutput=False)
    t2s = nc.declare_dram_parameter("t2s", [C, T2R, T2C], in_dram_dt, isOutput=False)
    n_gidx = (NIDX * (2 if GBATCH else 1)) // 16
    gidx = nc.declare_dram_parameter("gidx", [128, n_gidx], I16, isOutput=False)
    if V3:
        slotw = YB * 2 * X1W
    elif SPLIT_DESKEW:
        slotw = YB * SLOTW_SPLIT
    else:
        slotw = YB * 2 * NOUT
    out24 = nc.declare_dram_parameter("out24", [HSH // YB, C, slotw], F32, isOutput=True)

    do_mm = "mm" not in ABLATE
    do_evac = do_mm and "evac" not in ABLATE
    do_gather = do_evac and "gather" not in ABLATE
    do_outdma = do_gather and "outdma" not in ABLATE

    with ExitStack() as ctx:
        tc = ctx.enter_context(tile.TileContext(nc))
        inp = ctx.enter_context(tc.tile_pool(name="inp", bufs=1))
        bandp = ctx.enter_context(tc.tile_pool(name="band", bufs=BAND_BUFS))
        psump = ctx.enter_context(tc.tile_pool(name="psum", bufs=2, space="PSUM"))
        stgp = ctx.enter_context(tc.tile_pool(name="stg", bufs=2))

        in_dt = mm_dt if MM_DTYPE in ("bf16", "f32r") else F32
        t1sb = inp.tile([C, HSH, W], in_dt)
        t2sb = inp.tile([C, T2R, T2C], in_dt)
        gsb = inp.tile([128, n_gidx], I16)

        nc.sync.dma_start(gsb[:], gidx[:])

        rep_ctx = tc.For_i(0, REPEAT, 1) if REPEAT > 1 else None
        if rep_ctx is not None:
            ctx.enter_context(rep_ctx)

        # input DMAs (SWDGE casts on the fly for bf16); chunked so compute
        # can start before the full slab lands
        dma_in = nc.gpsimd.dma_start if MM_DTYPE == "bf16" else nc.sync.dma_start
        n_chunks = 8
        for ch in range(n_chunks) if "indma" not in ABLATE else []:
            r0, r1 = HSH * ch // n_chunks, HSH * (ch + 1) // n_chunks
            dma_in(t1sb[:, r0:r1, :], t1s[:, r0:r1, :])
            s0, s1 = T2R * ch // n_chunks, T2R * (ch + 1) // n_chunks
            dma_in(t2sb[:, s0:s1, :], t2s[:, s0:s1, :])

        for yb in range(HSH // YB):
            if V3:
                stg_shape = [C, YB * 2, X1W]
            elif SPLIT_DESKEW:
                stg_shape = [C, YB, SLOTW_SPLIT]
            else:
                stg_shape = [C, YB * 2, NOUT]
            stg = stgp.tile(stg_shape, F32, name="stg") if do_gather else None
            for y8 in range(YB):
                y = yb * YB + y8
                yband = (
                    bandp.tile([C, 2, NG, BAND_N], F32, name="yband")
                    if (GBATCH and do_evac)
                    else None
                )
                for xb in range(2):
                    ps = (
                        psump.tile([C, NG, 512], F32, name="ps") if do_mm else None
                    )
                    lhsT = t1sb[:, y, 128 * xb : 128 * xb + 128]
                    for g in range(NG) if ps is not None else []:
                        rhs = t2sb[
                            :,
                            y + NG * g : y + NG * g + NG,
                            128 * xb : 128 * xb + BAND_W,
                        ]
                        nc.tensor.matmul(
                            ps[:, g, 0:BAND_N], lhsT, rhs, start=True, stop=True
                        )
                    if V3 and do_evac and ps is not None and stg is not None:
                        # stg[32q+u, slot, (g,e,j)] = ps[32q+u, g, e*136+32q+j]
                        slot = y8 * 2 + xb
                        for q in range(4):
                            srcv = ps[32 * q : 32 * q + 32, :, 0:BAND_N].rearrange(
                                "p g (e w) -> p g e w", e=NG
                            )[:, :, :, 32 * q : 32 * q + 40]
                            dstv = stg[
                                32 * q : 32 * q + 32, slot, :
                            ].rearrange("p (g e j) -> p g e j", g=NG, e=NG)
                            if xb == 0:
                                nc.vector.tensor_copy(dstv, srcv)
                            else:
                                nc.scalar.copy(dstv, srcv)
                        band = None
                        continue_v3 = True
                    elif yband is not None:
                        band = yband[:, xb]
                    elif do_evac:
                        band = bandp.tile([C, NG, BAND_N], F32, name="band")
                    else:
                        band = None
                    if band is not None and not V3:
                        if xb == 0:
                            nc.vector.tensor_copy(band[:], ps[:, :, 0:BAND_N])
                        else:
                            nc.scalar.copy(band[:], ps[:, :, 0:BAND_N])
                    if not V3 and not GBATCH and stg is not None and band is not None:
                        if SPLIT_DESKEW and xb == 1:
                            # 32-partition-aligned rectangular extraction on DVE
                            # (quadrants 0-1) and ACT (quadrants 2-3):
                            # stg[32q+u, y8, 256+(g,e,j)] = band[32q+u, g, e*136+32q+j]
                            for q in range(4):
                                srcv = band[
                                    32 * q : 32 * q + 32, :, :
                                ].rearrange("p g (e w) -> p g e w", e=NG)[
                                    :, :, :, 32 * q : 32 * q + 40
                                ]
                                dstv = stg[
                                    32 * q : 32 * q + 32, y8, NOUT : NOUT + X1W
                                ].rearrange("p (g e j) -> p g e j", g=NG, e=NG)
                                if q < 2:
                                    nc.vector.tensor_copy(dstv, srcv)
                                else:
                                    nc.scalar.copy(dstv, srcv)
                        elif SPLIT_DESKEW:
                            nc.gpsimd.ap_gather(
                                stg[:, y8, 0:NOUT],
                                band.rearrange("p a b -> p (a b)"),
                                gsb[:],
                                channels=128,
                                num_elems=BAND_N * NG // GD,
                                d=GD,
                                num_idxs=NIDX,
                            )
                        else:
                            nc.gpsimd.ap_gather(
                                stg[:, y8 * 2 + xb, :],
                                band.rearrange("p a b -> p (a b)"),
                                gsb[:],
                                channels=128,
                                num_elems=BAND_N * NG // GD,
                                d=GD,
                                num_idxs=NIDX,
                            )
                if not V3 and GBATCH and stg is not None and yband is not None:
                    nc.gpsimd.ap_gather(
                        stg[:, y8 * 2 : y8 * 2 + 2, :].rearrange("p a b -> p (a b)"),
                        yband.rearrange("p a b c -> p (a b c)"),
                        gsb[:],
                        channels=128,
                        num_elems=2 * BAND_N * NG // GD,
                        d=GD,
                        num_idxs=2 * NIDX,
                    )
            if do_outdma:
                nc.scalar.dma_start(out24[yb], stg.rearrange("p a b -> p (a b)"))

    nc.finalize()
    return nc


_PROG_CACHE = {}


def get_program():
    key = (MM_DTYPE, REPEAT, GBATCH, GD, BAND_BUFS, SPLIT_DESKEW, V3, tuple(sorted(ABLATE)))
    if key not in _PROG_CACHE:
        _PROG_CACHE[key] = build_program()
    return _PROG_CACHE[key]


def make_in_maps(t1: np.ndarray, t2: np.ndarray):
    t1 = np.asarray(t1, dtype=np.float32)
    t2 = np.asarray(t2, dtype=np.float32)
    t2p = np.zeros((B, C, H + 2 * MD, W + 2 * MD), dtype=np.float32)
    t2p[:, :, MD : MD + H, MD : MD + W] = t2
    gidx = build_gidx()
    in_maps = []
    for core in range(8):
        b, h2 = divmod(core, 2)
        y0 = HSH * h2
        in_maps.append(
            {
                "t1s": np.ascontiguousarray(t1[b, :, y0 : y0 + HSH, :]),
                "t2s": np.ascontiguousarray(t2p[b, :, y0 : y0 + T2R, :]),
                "gidx": gidx,
            }
        )
    return in_maps


# host-side residual deskew index: I[xl, di, dj] = di*24 + (xl%16) + dj
_XL = np.arange(128)
_I = (
    np.arange(D)[None, :, None] * 24
    + (_XL % 16)[:, None, None]
    + np.arange(D)[None, None, :]
)  # [128, 9, 9]


_I40 = (
    np.arange(D)[None, :, None] * 40
    + (_XL % 32)[:, None, None]
    + np.arange(D)[None, None, :]
)  # [128, 9, 9] residual index for the 40-wide x-block-1 windows


def assemble_out(results) -> np.ndarray:
    out = np.empty((B, D * D, H, W), dtype=np.float32)
    if V3:
        idx = np.broadcast_to(
            _I40.reshape(1, 1, 1, 128, D * D), (HSH // YB, YB, 2, 128, D * D)
        )
        for core in range(8):
            b, h2 = divmod(core, 2)
            y0 = HSH * h2
            o = results[core]["out24"].reshape(HSH // YB, C, YB, 2, X1W)
            o = o.transpose(0, 2, 3, 1, 4)  # [yb, y8, xb, xl, w]
            g = np.take_along_axis(o, idx, axis=4)  # [yb, y8, xb, xl, 81]
            g = g.transpose(4, 0, 1, 2, 3)
            out[b, :, y0 : y0 + HSH, :] = g.reshape(D * D, HSH, W)
        return out
    if SPLIT_DESKEW:
        idx0 = np.broadcast_to(
            _I.reshape(1, 1, 1, 128, D * D), (HSH // YB, YB, 1, 128, D * D)
        )
        idx1 = np.broadcast_to(
            _I40.reshape(1, 1, 1, 128, D * D), (HSH // YB, YB, 1, 128, D * D)
        )
        for core in range(8):
            b, h2 = divmod(core, 2)
            y0 = HSH * h2
            o = results[core]["out24"].reshape(HSH // YB, C, YB, SLOTW_SPLIT)
            o = o.transpose(0, 2, 1, 3)[:, :, None, :, :]  # [yb, y8, 1, xl, w]
            g0 = np.take_along_axis(o[..., 0:NOUT], idx0, axis=4)
            g1 = np.take_along_axis(o[..., NOUT:], idx1, axis=4)
            g = np.concatenate([g0, g1], axis=2)  # [yb, y8, xb, xl, 81]
            g = g.transpose(4, 0, 1, 2, 3)  # [81, yb, y8, xb, xl]
            out[b, :, y0 : y0 + HSH, :] = g.reshape(D * D, HSH, W)
        return out
    idx = np.broadcast_to(
        _I.reshape(1, 1, 1, 128, D * D), (HSH // YB, YB, 2, 128, D * D)
    )
    for core in range(8):
        b, h2 = divmod(core, 2)
        y0 = HSH * h2
        o = results[core]["out24"].reshape(HSH // YB, C, YB, 2, NOUT)
        o = o.transpose(0, 2, 3, 1, 4)  # [yb, y8, xb, xl, i]
        g = np.take_along_axis(o, idx, axis=4)  # [yb, y8, xb, xl, 81]
        g = g.transpose(4, 0, 1, 2, 3)  # [81, yb, y8, xb, xl]
        out[b, :, y0 : y0 + HSH, :] = g.reshape(D * D, HSH, W)
    return out


def run(t1: np.ndarray, t2: np.ndarray, trace: bool = False, **kw):
    nc = get_program()
    in_maps = make_in_maps(t1, t2)
    res = run_bass_kernel_spmd(nc, in_maps, list(range(8)), trace=trace, **kw)
    return assemble_out(res.results), res


def kernel(t1: np.ndarray, t2: np.ndarray) -> np.ndarray:
    return run(t1, t2)[0]


if __name__ == "__main__":
    t1 = np.random.randn(B, C, H, W).astype(np.float32)
    t2 = np.random.randn(B, C, H, W).astype(np.float32)
    out = kernel(t1, t2)
    print(out.shape, out.dtype)


# revision 27
# speedup vs baseline: 1.0093x; 1.0093x over previous
"""
Trainium2 Bass kernel for nn_BaseDecoder (9x9 local cost volume / spatial
correlation, kernel_size=1):

    out[b, di*9+dj, y, x] = sum_c t1[b,c,y,x] * t2p[b,c,y+di,x+dj]

t1/t2: [4, 128, 128, 256] f32, out: [4, 81, 128, 256] f32, zero-padded t2.

Strategy
--------
8 cores = (batch 4) x (H halves 2), fully data parallel; each core gets its
t1 shard [128c, 64y, 256x] and a zero-padded t2 slab [128c, 72y, 264x]
(4-row/4-col halo baked in on host), so no collectives are needed.

Per (y, x-block-of-128): channels live on partitions, so the only engine
that can produce the 81 shifted dot-products at speed is the PE via a
*banded* matmul psum[x, w] = sum_c t1[c, x0+x] * t2slab[c, y+di, x0+w],
done as 3 float32r matmuls of N=3*136 (3 'di' rows each; one PSUM bank
each; f32r streams 1 cyc/col at N>=256 vs 4 for fp32, rel err ~1.5e-4).
The useful outputs are the 9 diagonals psum[x, x+dj] of each 128x136 band
-- inherently non-rectangular, which no lockstep engine (PE/DVE/ACT/DMA)
can extract.

The band is evacuated PSUM->SBUF on VectorE (x-block 0) and ScalarE
(x-block 1) in parallel, then GPSIMD `ap_gather` -- the one engine with
per-Q7-core addressing -- extracts the 16-partition-aligned sub-band
out24[x, di, j] = band[x, di, 16*(x//16) + j], j in [0,24), as d=8-element
blocks (per-index cost dominates: d=1 measured 3.4x slower).  This shrinks
1224 -> 216 useful floats per partition per (y, x-block).  Results DMA out
via the ScalarE HWDGE ring (inputs use the sync ring).  The remaining
within-core skew (j = (x%16)+dj) is a cheap numpy take_along_axis on host.

Deskew split (SPLIT_DESKEW=1, the shipped config): Pool `ap_gather`
handles x-block 0; for x-block 1 the 32-partition-aligned 40-wide windows
ARE rectangular per partition-quadrant, so VectorE/ScalarE extract them
with 4 sliced copies (host residual (x%32)+dj there).  Measured (R-delta
slope, 8-core SPMD on HW): ~217 us/core/sweep; HBM roofline for the
23 MB/core I/O is ~64 us.  Tried and rejected: extracting the quadrant
windows directly from PSUM with 8 small ops/row (KERNEL_V3=1) measures
286 us -- the TRN2 PSUM read-write bubble errata penalizes many small
PSUM-source ops, so the single big evacuation wins.  Remaining wall:
GPSIMD gather + imperfect Pool/DVE overlap (shared SBUF port); going
lower needs custom Q7 ucode.
"""

import os
import sys

sys.path.insert(0, "/opt/trn_rl_repo")

from contextlib import ExitStack

import numpy as np

import concourse.bacc as bacc
import concourse.bass as bass
import concourse.mybir as mybir
import concourse.tile as tile
from concourse.bass_utils import run_bass_kernel_spmd

MD = 4
D = 9  # patch size (9x9 displacements)
B, C, H, W = 4, 128, 128, 256
HSH = H // 2  # 64 rows per shard
T2R = HSH + 2 * MD  # 72 t2 slab rows
T2C = W + 2 * MD  # 264 t2 slab cols
NG = 3  # di-groups (3 di each)
BAND_W = 136  # x' window width per di (128 + 8)
BAND_N = NG * BAND_W  # 408 = matmul N (one PSUM bank)
GD = int(os.environ.get("KERNEL_GATHER_D", "8"))  # ap_gather inner block size
assert GD in (1, 8)
# with d=8: 27 useful block-indices (9 di x 3 blocks of 8), padded to 32 (%16)
# with d=1: 216 useful element-indices, padded to 224
NBLK = (D * 24) // GD  # useful indices
NIDX = 224 if GD == 1 else 32  # padded index count (%16 == 0)
NOUT = NIDX * GD  # gathered floats per partition per x-block
X1W = D * 40  # 360: 40-wide 32-aligned windows for the split-deskew x-block 1
SLOTW_SPLIT = NOUT + X1W  # 616
NUSE = D * 24  # 216 useful floats
YB = int(os.environ.get("KERNEL_YB", "8"))  # y rows per output DMA batch

F32 = mybir.dt.float32
I16 = mybir.dt.int16

# matmul input dtype: "f32" (exact, 4 cyc/col), "f32r" (fast fp32 path),
# "bf16" (fast, lossy)
MM_DTYPE = os.environ.get("KERNEL_MM_DTYPE", "f32r")
# internal whole-kernel repeat count (for HW timing via differencing)
REPEAT = int(os.environ.get("KERNEL_REPEAT", "1"))
# comma list of stages to drop, for cost-model ablation: mm,evac,gather,outdma,indma
ABLATE = set(filter(None, os.environ.get("KERNEL_ABLATE", "").split(",")))
# 1 = one ap_gather per y (both x-blocks from one band tile); 0 = per x-block
GBATCH = int(os.environ.get("KERNEL_GBATCH", "0"))
BAND_BUFS = int(os.environ.get("KERNEL_BAND_BUFS", "2"))
# 1 = Pool ap_gather deskews only x-block 0; DVE/ACT extract x-block 1 via
# 16-partition-sliced rectangular group copies (they have slack; Pool is the wall)
SPLIT_DESKEW = int(os.environ.get("KERNEL_SPLIT_DESKEW", "1"))
# 1 = v3: extract 32-aligned 40-wide quadrant windows DIRECTLY from PSUM on
# DVE(xb0)/ACT(xb1); no full-band evacuation, no GPSIMD at all
V3 = int(os.environ.get("KERNEL_V3", "0"))
# 1 = all four x-block-1 quadrant extracts on ScalarE (minimize DVE load,
# which shares an SBUF port with the Pool gather)
XTRACT_ACT = int(os.environ.get("KERNEL_XTRACT_ACT", "0"))


def build_gidx() -> np.ndarray:
    """Per-Q7-core gather index lists for ap_gather, stored 'wrapped':
    unwrapped[i] = idxs[16k + i%16, i//16] for core k.  Index values are in
    units of GD-element blocks.  With GBATCH, indices cover both x-blocks
    (xb-major) of a [2, NG, BAND_N] band tile."""
    nidx_tot = NIDX * (2 if GBATCH else 1)
    idx = np.zeros((128, nidx_tot // 16), dtype=np.int16)
    for k in range(8):
        for i in range(nidx_tot):
            xb, ih = divmod(i, NIDX) if GBATCH else (0, i)
            if ih < NBLK:
                di, m = divmod(ih, 24 // GD)
                g, e = divmod(di, NG)
                val = (xb * NG * BAND_N + g * BAND_N + e * BAND_W + 16 * k) // GD + m
            else:
                val = 0
            idx[16 * k + (i % 16), i // 16] = val
    return idx


def build_program():
    nc = bacc.Bacc("TRN2")

    if MM_DTYPE == "bf16":
        mm_dt = mybir.dt.bfloat16
    elif MM_DTYPE == "f32r":
        mm_dt = mybir.dt.float32r
    else:
        mm_dt = F32
    in_dram_dt = mm_dt if MM_DTYPE == "f32r" else F32
    t1s = nc.declare_dram_parameter("t1s", [C, HSH, W], in_dram_dt, isOutput=False)
    t2s = nc.declare_dram_parameter("t2s", [C, T2R, T2C], in_dram_dt, isOutput=False)
    n_gidx = (NIDX * (2 if GBATCH else 1)) // 16
    gidx = nc.declare_dram_parameter("gidx", [128, n_gidx], I16, isOutput=False)
    if V3:
        slotw = YB * 2 * X1W
    elif SPLIT_DESKEW:
        slotw = YB * SLOTW_SPLIT
    else:
        slotw = YB * 2 * NOUT
    out24 = nc.declare_dram_parameter("out24", [HSH // YB, C, slotw], F32, isOutput=True)

    do_mm = "mm" not in ABLATE
    do_evac = do_mm and "evac" not in ABLATE
    do_gather = do_evac and "gather" not in ABLATE
    do_outdma = do_gather and "outdma" not in ABLATE

    with ExitStack() as ctx:
        tc = ctx.enter_context(tile.TileContext(nc))
        inp = ctx.enter_context(tc.tile_pool(name="inp", bufs=1))
        bandp = ctx.enter_context(tc.tile_pool(name="band", bufs=BAND_BUFS))
        psump = ctx.enter_context(tc.tile_pool(name="psum", bufs=2, space="PSUM"))
        stgp = ctx.enter_context(tc.tile_pool(name="stg", bufs=2))

        in_dt = mm_dt if MM_DTYPE in ("bf16", "f32r") else F32
        t1sb = inp.tile([C, HSH, W], in_dt)
        t2sb = inp.tile([C, T2R, T2C], in_dt)
        gsb = inp.tile([128, n_gidx], I16)

        nc.sync.dma_start(gsb[:], gidx[:])

        rep_ctx = tc.For_i(0, REPEAT, 1) if REPEAT > 1 else None
        if rep_ctx is not None:
            ctx.enter_context(rep_ctx)

        # input DMAs (SWDGE casts on the fly for bf16); chunked so compute
        # can start before the full slab lands
        dma_in = nc.gpsimd.dma_start if MM_DTYPE == "bf16" else nc.sync.dma_start
        n_chunks = 8
        for ch in range(n_chunks) if "indma" not in ABLATE else []:
            r0, r1 = HSH * ch // n_chunks, HSH * (ch + 1) // n_chunks
            dma_in(t1sb[:, r0:r1, :], t1s[:, r0:r1, :])
            s0, s1 = T2R * ch // n_chunks, T2R * (ch + 1) // n_chunks
            dma_in(t2sb[:, s0:s1, :], t2s[:, s0:s1, :])

        for yb in range(HSH // YB):
            if V3:
                stg_shape = [C, YB * 2, X1W]
            elif SPLIT_DESKEW:
                stg_shape = [C, YB, SLOTW_SPLIT]
            else:
                stg_shape = [C, YB * 2, NOUT]
            stg = stgp.tile(stg_shape, F32, name="stg") if do_gather else None
            for y8 in range(YB):
                y = yb * YB + y8
                yband = (
                    bandp.tile([C, 2, NG, BAND_N], F32, name="yband")
                    if (GBATCH and do_evac)
                    else None
                )
                for xb in range(2):
                    ps = (
                        psump.tile([C, NG, 512], F32, name="ps") if do_mm else None
                    )
                    lhsT = t1sb[:, y, 128 * xb : 128 * xb + 128]
                    for g in range(NG) if ps is not None else []:
                        rhs = t2sb[
                            :,
                            y + NG * g : y + NG * g + NG,
                            128 * xb : 128 * xb + BAND_W,
                        ]
                        nc.tensor.matmul(
                            ps[:, g, 0:BAND_N], lhsT, rhs, start=True, stop=True
                        )
                    if V3 and do_evac and ps is not None and stg is not None:
                        # stg[32q+u, slot, (g,e,j)] = ps[32q+u, g, e*136+32q+j]
                        slot = y8 * 2 + xb
                        for q in range(4):
                            srcv = ps[32 * q : 32 * q + 32, :, 0:BAND_N].rearrange(
                                "p g (e w) -> p g e w", e=NG
                            )[:, :, :, 32 * q : 32 * q + 40]
                            dstv = stg[
                                32 * q : 32 * q + 32, slot, :
                            ].rearrange("p (g e j) -> p g e j", g=NG, e=NG)
                            if xb == 0:
                                nc.vector.tensor_copy(dstv, srcv)
                            else:
                                nc.scalar.copy(dstv, srcv)
                        band = None
                        continue_v3 = True
                    elif yband is not None:
                        band = yband[:, xb]
                    elif do_evac:
                        band = bandp.tile([C, NG, BAND_N], F32, name="band")
                    else:
                        band = None
                    if band is not None and not V3:
                        if xb == 0:
                            nc.vector.tensor_copy(band[:], ps[:, :, 0:BAND_N])
                        else:
                            nc.scalar.copy(band[:], ps[:, :, 0:BAND_N])
                    if not V3 and not GBATCH and stg is not None and band is not None:
                        if SPLIT_DESKEW and xb == 1:
                            # 32-partition-aligned rectangular extraction on DVE
                            # (quadrants 0-1) and ACT (quadrants 2-3):
                            # stg[32q+u, y8, 256+(g,e,j)] = band[32q+u, g, e*136+32q+j]
                            for q in range(4):
                                srcv = band[
                                    32 * q : 32 * q + 32, :, :
                                ].rearrange("p g (e w) -> p g e w", e=NG)[
                                    :, :, :, 32 * q : 32 * q + 40
                                ]
                                dstv = stg[
                                    32 * q : 32 * q + 32, y8, NOUT : NOUT + X1W
                                ].rearrange("p (g e j) -> p g e j", g=NG, e=NG)
                                if q < 2 and not XTRACT_ACT:
                                    nc.vector.tensor_copy(dstv, srcv)
                                else:
                                    nc.scalar.copy(dstv, srcv)
                        elif SPLIT_DESKEW:
                            nc.gpsimd.ap_gather(
                                stg[:, y8, 0:NOUT],
                                band.rearrange("p a b -> p (a b)"),
                                gsb[:],
                                channels=128,
                                num_elems=BAND_N * NG // GD,
                                d=GD,
                                num_idxs=NIDX,
                            )
                        else:
                            nc.gpsimd.ap_gather(
                                stg[:, y8 * 2 + xb, :],
                                band.rearrange("p a b -> p (a b)"),
                                gsb[:],
                                channels=128,
                                num_elems=BAND_N * NG // GD,
                                d=GD,
                                num_idxs=NIDX,
                            )
                if not V3 and GBATCH and stg is not None and yband is not None:
                    nc.gpsimd.ap_gather(
                        stg[:, y8 * 2 : y8 * 2 + 2, :].rearrange("p a b -> p (a b)"),
                        yband.rearrange("p a b c -> p (a b c)"),
                        gsb[:],
                        channels=128,
                        num_elems=2 * BAND_N * NG // GD,
                        d=GD,
                        num_idxs=2 * NIDX,
                    )
            if do_outdma:
                nc.scalar.dma_start(out24[yb], stg.rearrange("p a b -> p (a b)"))

    nc.finalize()
    return nc


_PROG_CACHE = {}


def get_program():
    key = (MM_DTYPE, REPEAT, GBATCH, GD, BAND_BUFS, SPLIT_DESKEW, V3, XTRACT_ACT, tuple(sorted(ABLATE)))
    if key not in _PROG_CACHE:
        _PROG_CACHE[key] = build_program()
    return _PROG_CACHE[key]


def make_in_maps(t1: np.ndarray, t2: np.ndarray):
    t1 = np.asarray(t1, dtype=np.float32)
    t2 = np.asarray(t2, dtype=np.float32)
    t2p = np.zeros((B, C, H + 2 * MD, W + 2 * MD), dtype=np.float32)
    t2p[:, :, MD : MD + H, MD : MD + W] = t2
    gidx = build_gidx()
    in_maps = []
    for core in range(8):
        b, h2 = divmod(core, 2)
        y0 = HSH * h2
        in_maps.append(
            {
                "t1s": np.ascontiguousarray(t1[b, :, y0 : y0 + HSH, :]),
                "t2s": np.ascontiguousarray(t2p[b, :, y0 : y0 + T2R, :]),
                "gidx": gidx,
            }
        )
    return in_maps


# host-side residual deskew index: I[xl, di, dj] = di*24 + (xl%16) + dj
_XL = np.arange(128)
_I = (
    np.arange(D)[None, :, None] * 24
    + (_XL % 16)[:, None, None]
    + np.arange(D)[None, None, :]
)  # [128, 9, 9]


_I40 = (
    np.arange(D)[None, :, None] * 40
    + (_XL % 32)[:, None, None]
    + np.arange(D)[None, None, :]
)  # [128, 9, 9] residual index for the 40-wide x-block-1 windows


def assemble_out(results) -> np.ndarray:
    out = np.empty((B, D * D, H, W), dtype=np.float32)
    if V3:
        idx = np.broadcast_to(
            _I40.reshape(1, 1, 1, 128, D * D), (HSH // YB, YB, 2, 128, D * D)
        )
        for core in range(8):
            b, h2 = divmod(core, 2)
            y0 = HSH * h2
            o = results[core]["out24"].reshape(HSH // YB, C, YB, 2, X1W)
            o = o.transpose(0, 2, 3, 1, 4)  # [yb, y8, xb, xl, w]
            g = np.take_along_axis(o, idx, axis=4)  # [yb, y8, xb, xl, 81]
            g = g.transpose(4, 0, 1, 2, 3)
            out[b, :, y0 : y0 + HSH, :] = g.reshape(D * D, HSH, W)
        return out
    if SPLIT_DESKEW:
        idx0 = np.broadcast_to(
            _I.reshape(1, 1, 1, 128, D * D), (HSH // YB, YB, 1, 128, D * D)
        )
        idx1 = np.broadcast_to(
            _I40.reshape(1, 1, 1, 128, D * D), (HSH // YB, YB, 1, 128, D * D)
        )
        for core in range(8):
            b, h2 = divmod(core, 2)
            y0 = HSH * h2
            o = results[core]["out24"].reshape(HSH // YB, C, YB, SLOTW_SPLIT)
            o = o.transpose(0, 2, 1, 3)[:, :, None, :, :]  # [yb, y8, 1, xl, w]
            g0 = np.take_along_axis(o[..., 0:NOUT], idx0, axis=4)
            g1 = np.take_along_axis(o[..., NOUT:], idx1, axis=4)
            g = np.concatenate([g0, g1], axis=2)  # [yb, y8, xb, xl, 81]
            g = g.transpose(4, 0, 1, 2, 3)  # [81, yb, y8, xb, xl]
            out[b, :, y0 : y0 + HSH, :] = g.reshape(D * D, HSH, W)
        return out
    idx = np.broadcast_to(
        _I.reshape(1, 1, 1, 128, D * D), (HSH // YB, YB, 2, 128, D * D)
    )
    for core in range(8):
        b, h2 = divmod(core, 2)
        y0 = HSH * h2
        o = results[core]["out24"].reshape(HSH // YB, C, YB, 2, NOUT)
        o = o.transpose(0, 2, 3, 1, 4)  # [yb, y8, xb, xl, i]
        g = np.take_along_axis(o, idx, axis=4)  # [yb, y8, xb, xl, 81]
        g = g.transpose(4, 0, 1, 2, 3)  # [81, yb, y8, xb, xl]
        out[b, :, y0 : y0 + HSH, :] = g.reshape(D * D, HSH, W)
    return out


def run(t1: np.ndarray, t2: np.ndarray, trace: bool = False, **kw):
    nc = get_program()
    in_maps = make_in_maps(t1, t2)
    res = run_bass_kernel_spmd(nc, in_maps, list(range(8)), trace=trace, **kw)
    return assemble_out(res.results), res


def kernel(t1: np.ndarray, t2: np.ndarray) -> np.ndarray:
    return run(t1, t2)[0]


if __name__ == "__main__":
    t1 = np.random.randn(B, C, H, W).astype(np.float32)
    t2 = np.random.randn(B, C, H, W).astype(np.float32)
    out = kernel(t1, t2)
    print(out.shape, out.dtype)
